# revision 9
# baseline (speedup 1.0000x reference)
"""ColAttention TRN2 kernel: out = gamma * colattn(x) + x.

Sharding: width. Core k gets x[:, :, :, 16k:16(k+1)] (host slice, bf16).
Per core: 8 batches x 16 width columns = 128 independent attention
problems over h=128.

v3 pipeline, per (b, w) column (all engines kept busy, PE never waits):
  S^T(j,i) = K_w.T Q_w  directly via matmul (no PE transpose, no PSUM
             round-trip: exp(S^T) on ACT writes attn^T straight to SBUF)
  V^T_w (h,c) = x_w.T @ (gamma*Wv).T   (bf16, or fp8 DoubleRow pairs)
  attn^T = exp(S^T) -> bf16 SBUF (unnormalized; no max-subtraction
           needed: scores in +-50, exp stays in f32/bf16 range)
  delta^T(i,c) = attn^T.T @ V^T  as ONE 512-col matmul (ats stationary)
  sums(i) = attn^T.T @ ones     (1-col matmul into per-batch PSUM tile)
  delta copied to SBUF bf16 (ACT) and DMA'd per column (Pool queue)
Device ships unnormalized delta^T (bf16) + row sums (f32); host does
out = x + delta/sums (+ gamma*bv), keeping the residual exact in f32.
"""

import numpy as np
import ml_dtypes

import concourse.bass as bass
from concourse import bacc, mybir
from concourse.tile import TileContext
from concourse.bass_utils import run_bass_kernel_spmd

f32 = mybir.dt.float32
bf16 = mybir.dt.bfloat16
fp8 = mybir.dt.float8e4
AF = mybir.ActivationFunctionType
PM = mybir.MatmulPerfMode

N_CORES = 8
B, C, H, W = 8, 512, 128, 128
WT = W // N_CORES          # 16 w-columns per core
DQ = 64
NCH = C // 128             # 4 c-chunks

FP8_V = True               # fp8 DoubleRow for the V^T projection
TRACE = False              # set True from test.py for profiling
LAST_RESULTS = None


def _build(bqk_is_zero: bool):
    nc = bacc.Bacc("TRN2", num_devices=N_CORES, debug=False)

    x_d = nc.dram_tensor("x", (B, C, H, WT), bf16, kind="ExternalInput")
    wqk_d = nc.dram_tensor("wqkT", (C, 128), bf16, kind="ExternalInput")
    bqk_d = nc.dram_tensor("bqk", (128, 1), f32, kind="ExternalInput")
    wv_d = nc.dram_tensor("wvT", (C, C), fp8 if FP8_V else bf16,
                          kind="ExternalInput")
    if FP8_V:
        xf_d = nc.dram_tensor("xf", (B, C, H, WT), fp8, kind="ExternalInput")
    out_d = nc.dram_tensor("out", (B, WT, H, C), bf16, kind="ExternalOutput")
    sums_d = nc.dram_tensor("sums", (B, H, WT), f32, kind="ExternalOutput")
    ones_d = nc.inline_tensor(np.ones((128, 1), dtype=ml_dtypes.bfloat16),
                              name="ones128")

    xa = x_d.ap()
    oa = out_d.ap()
    sa = sums_d.ap()

    with TileContext(nc) as tc:
        with (
            tc.tile_pool(name="const", bufs=1) as cpool,
            tc.tile_pool(name="xs", bufs=2) as xspool,
            tc.tile_pool(name="qk", bufs=2) as qkpool,
            tc.tile_pool(name="small", bufs=3) as spool,
            tc.tile_pool(name="pqk", bufs=1, space="PSUM") as pqk,
            tc.tile_pool(name="pvt", bufs=2, space="PSUM") as pvt,
            tc.tile_pool(name="psct", bufs=2, space="PSUM") as psct,
            tc.tile_pool(name="pav", bufs=2, space="PSUM") as pav,
            tc.tile_pool(name="psm", bufs=1, space="PSUM") as psm,
        ):
            # ---- constants ----
            wqk_sb = cpool.tile([128, 128 * NCH], bf16, name="wqk_sb")
            for ci in range(NCH):
                nc.sync.dma_start(wqk_sb[:, ci * 128:(ci + 1) * 128],
                                  wqk_d.ap()[ci * 128:(ci + 1) * 128, :])
            ones_sb = cpool.tile([128, 1], bf16, name="ones_sb")
            nc.sync.dma_start(ones_sb[:], ones_d.ap())
            bqk_sb = cpool.tile([128, 1], f32, name="bqk_sb")
            nc.sync.dma_start(bqk_sb[:], bqk_d.ap())
            wv_sb = cpool.tile([128, 512 * NCH], fp8 if FP8_V else bf16,
                               name="wv_sb")
            for ci in range(NCH):
                nc.gpsimd.dma_start(wv_sb[:, ci * 512:(ci + 1) * 512],
                                    wv_d.ap()[ci * 128:(ci + 1) * 128, :])

            for b in range(B):
                # ---- batch prologue: hoisted into previous batch's w-loop ----
                with tc.high_priority(offset=0 if b == 0 else 200):
                    # load slab (4 chunks, contiguous 512 KiB each)
                    xs = xspool.tile([128, NCH * H * WT], bf16, tag="xs",
                                     name=f"xs{b}")
                    xs4 = xs[:].rearrange("p (c h w) -> p c h w", c=NCH, w=WT)
                    for ci in range(NCH):
                        nc.sync.dma_start(xs4[:, ci], xa[b, ci * 128:(ci + 1) * 128])
                    if FP8_V:
                        xf = xspool.tile([128, NCH * H * WT], fp8, tag="xf",
                                         name=f"xf{b}")
                        xf4 = xf[:].rearrange("p (c h w) -> p c h w", c=NCH, w=WT)
                        for ci in range(NCH):
                            nc.gpsimd.dma_start(xf4[:, ci],
                                                xf_d.ap()[b, ci * 128:(ci + 1) * 128])

                    # QK projection: full (h,w) range, n-tiles of 512
                    qk_sb = qkpool.tile([128, H * WT], bf16, tag="qk", name=f"qk{b}")
                    ks = qkpool.tile([64, H * WT], bf16, tag="ks", name=f"ks{b}")
                    for nt in range(H * WT // 512):
                        qkp = pqk.tile([128, 512], f32, tag="qkp")
                        for ci in range(NCH):
                            nc.tensor.matmul(
                                qkp[:],
                                wqk_sb[:, ci * 128:(ci + 1) * 128],
                                xs[:, ci * 2048 + nt * 512: ci * 2048 + (nt + 1) * 512],
                                start=(ci == 0), stop=(ci == NCH - 1))
                        if not bqk_is_zero:
                            nc.scalar.activation(qk_sb[:, nt * 512:(nt + 1) * 512],
                                                 qkp[:], AF.Identity, bias=bqk_sb[:])
                        else:
                            nc.vector.tensor_copy(qk_sb[:, nt * 512:(nt + 1) * 512],
                                                  qkp[:])
                        # K rows 64:128 -> partitions 0:63 (scores needs same base)
                        nc.sync.dma_start(ks[:, nt * 512:(nt + 1) * 512],
                                          qk_sb[64:128, nt * 512:(nt + 1) * 512])

                    sums_ps = psm.tile([128, WT], f32, tag="sums", name=f"sm{b}")
                qk3 = qk_sb[:].rearrange("p (h w) -> p h w", w=WT)
                ks3 = ks[:].rearrange("p (h w) -> p h w", w=WT)

                def emit_tail(prev):
                    pats, pv, pw = prev
                    # delta^T(i, c) in one 512-col matmul; ats stationary
                    av = pav.tile([128, 512], f32, tag="av")
                    nc.tensor.matmul(av[:], pats[:], pv[:], start=True, stop=True)
                    # row sums: 1-col matmul, same stationary weights
                    nc.tensor.matmul(sums_ps[:, pw:pw + 1], pats[:], ones_sb[:],
                                     start=True, stop=True, skip_group_check=True)
                    dcol = spool.tile([128, 512], bf16, tag="dcol")
                    nc.scalar.activation(dcol[:], av[:], AF.Identity)
                    nc.gpsimd.dma_start(oa[b, pw], dcol[:])

                prev = None
                for w in range(WT):
                    # ---- S^T(j,i) = K_w.T Q_w, bf16, k=64 ----
                    sct = psct.tile([128, 128], f32, tag="sct")
                    nc.tensor.matmul(sct[:], ks3[:, :, w], qk3[0:64, :, w],
                                     start=True, stop=True)

                    # ---- V^T_w (h, c) ----
                    vt = pvt.tile([128, 512], f32, tag="vt")
                    if FP8_V:
                        for ci in range(2):
                            nc.tensor.matmul(
                                vt[:], xf4[:, 2 * ci:2 * ci + 2, :, w],
                                wv_sb[:].rearrange("p (c n) -> p c n", n=512)
                                     [:, 2 * ci:2 * ci + 2],
                                start=(ci == 0), stop=(ci == 1),
                                perf_mode=PM.DoubleRow)
                    else:
                        for ci in range(NCH):
                            nc.tensor.matmul(vt[:], xs4[:, ci, :, w],
                                             wv_sb[:, ci * 512:(ci + 1) * 512],
                                             start=(ci == 0), stop=(ci == NCH - 1))

                    # ---- attn^T (unnormalized) = exp(S^T), straight to SBUF ----
                    ats = spool.tile([128, 128], bf16, tag="ats")
                    nc.scalar.activation(ats[:], sct[:], AF.Exp)

                    # ---- V^T to SBUF bf16 ----
                    v_sb = spool.tile([128, 512], bf16, tag="v_sb")
                    nc.vector.tensor_copy(v_sb[:], vt[:])

                    # ---- AV + sums + store for previous column (PE stays fed) ----
                    if prev is not None:
                        emit_tail(prev)
                    prev = (ats, v_sb, w)

                emit_tail(prev)
                sums_sb = spool.tile([128, WT], f32, tag="sums_sb")
                nc.vector.tensor_copy(sums_sb[:], sums_ps[:])
                nc.sync.dma_start(sa[b], sums_sb[:])

    nc.compile()
    return nc


def kernel(x, Wq, bq, Wk, bk, Wv, bv, gamma):
    global LAST_RESULTS
    x = np.ascontiguousarray(np.asarray(x, dtype=np.float32))
    Wq = np.asarray(Wq, dtype=np.float32)
    bq = np.asarray(bq, dtype=np.float32)
    Wk = np.asarray(Wk, dtype=np.float32)
    bk = np.asarray(bk, dtype=np.float32)
    Wv = np.asarray(Wv, dtype=np.float32)
    bv = np.asarray(bv, dtype=np.float32)
    g = float(np.asarray(gamma, dtype=np.float32).reshape(-1)[0])

    nc = _build(not (np.any(bq) or np.any(bk)))

    wqkT = np.ascontiguousarray(
        np.concatenate([Wq, Wk], axis=0).T).astype(ml_dtypes.bfloat16)  # (C, 128)
    bqk = np.concatenate([bq, bk], axis=0).reshape(128, 1)
    wv_dt = ml_dtypes.float8_e4m3 if FP8_V else ml_dtypes.bfloat16
    wvT = np.ascontiguousarray((g * Wv).T).astype(wv_dt)                # (C, C)
    xb = x.astype(ml_dtypes.bfloat16)
    if FP8_V:
        xf = x.astype(ml_dtypes.float8_e4m3)

    in_maps = []
    for k in range(N_CORES):
        m = {
            "x": np.ascontiguousarray(xb[:, :, :, k * WT:(k + 1) * WT]),
            "wqkT": wqkT,
            "bqk": bqk,
            "wvT": wvT,
        }
        if FP8_V:
            m["xf"] = np.ascontiguousarray(xf[:, :, :, k * WT:(k + 1) * WT])
        in_maps.append(m)

    res = run_bass_kernel_spmd(nc, in_maps, core_ids=list(range(N_CORES)),
                               trace=TRACE)
    LAST_RESULTS = res

    gbv = (g * bv).reshape(1, C, 1, 1).astype(np.float32)
    out = np.empty((B, C, H, W), dtype=np.float32)
    for k in range(N_CORES):
        d = np.asarray(res.results[k]["out"]).astype(np.float32)  # (B,WT,H,C)
        s = np.asarray(res.results[k]["sums"])                    # (B,H,WT)
        rr = (1.0 / s)[:, None, :, :]                             # (B,1,H,WT)
        out[:, :, :, k * WT:(k + 1) * WT] = (
            x[:, :, :, k * WT:(k + 1) * WT]
            + d.transpose(0, 3, 2, 1) * rr + gbv)
    return out


# revision 13
# speedup vs baseline: 1.1726x; 1.1726x over previous
"""ColAttention TRN2 kernel: out = gamma * colattn(x) + x.

Sharding: width. Core k gets x[:, :, :, 16k:16(k+1)] (host slice, bf16).
Per core: 8 batches x 16 width columns = 128 independent attention
problems over h=128.

v4 pipeline, per (b, w) column (PE-limited; all-bf16 — fp8 DoubleRow
measured SLOWER here: it trips the power throttle to ~64% util):
  S^T(j,i) = K_w.T Q_w  directly via matmul (no PE transpose; exp(S^T)
             on ACT writes attn^T straight to SBUF)
  V^T_w (h,c) = x_w.T @ (gamma*Wv).T   (4 bf16 matmuls, 512-col streams)
  attn^T = exp(S^T) -> bf16 SBUF (unnormalized; scores are in +-50 so
           exp stays in f32/bf16 range without max-subtraction)
  delta^T(i,c) = attn^T.T @ [V^T | ones]  as two matmuls (256/257 cols);
           the ones column makes delta col 512 the softmax row-sums —
           no separate reduction instruction anywhere
  delta pairs (2 columns) DMA'd from a 1024-col staging tile (Pool queue)
Device ships unnormalized delta^T (bf16) + row sums (f32); host does
out = x + delta/sums (+ gamma*bv), keeping the residual exact in f32.
"""

import numpy as np
import ml_dtypes

import concourse.bass as bass
from concourse import bacc, mybir
from concourse.tile import TileContext
from concourse.bass_utils import run_bass_kernel_spmd

f32 = mybir.dt.float32
bf16 = mybir.dt.bfloat16
AF = mybir.ActivationFunctionType

N_CORES = 8
B, C, H, W = 8, 512, 128, 128
WT = W // N_CORES          # 16 w-columns per core
DQ = 64
NCH = C // 128             # 4 c-chunks

TRACE = False              # set True from test.py for profiling
LAST_RESULTS = None


def _build(bqk_is_zero: bool):
    nc = bacc.Bacc("TRN2", num_devices=N_CORES, debug=False)

    x_d = nc.dram_tensor("x", (B, C, H, WT), bf16, kind="ExternalInput")
    wqk_d = nc.dram_tensor("wqkT", (C, 128), bf16, kind="ExternalInput")
    bqk_d = nc.dram_tensor("bqk", (128, 1), f32, kind="ExternalInput")
    wv_d = nc.dram_tensor("wvT", (C, C), bf16, kind="ExternalInput")
    out_d = nc.dram_tensor("out", (B, H, WT, C), bf16, kind="ExternalOutput")
    sums_d = nc.dram_tensor("sums", (B, H, WT), f32, kind="ExternalOutput")

    xa = x_d.ap()
    oa = out_d.ap()
    sa = sums_d.ap()

    with TileContext(nc) as tc:
        with (
            tc.tile_pool(name="const", bufs=1) as cpool,
            tc.tile_pool(name="xs", bufs=2) as xspool,
            tc.tile_pool(name="qk", bufs=2) as qkpool,
            tc.tile_pool(name="small", bufs=3) as spool,
            tc.tile_pool(name="dc", bufs=2) as dcpool,
            tc.tile_pool(name="pqk", bufs=1, space="PSUM") as pqk,
            tc.tile_pool(name="pvt", bufs=2, space="PSUM") as pvt,
            tc.tile_pool(name="psct", bufs=1, space="PSUM") as psct,
            tc.tile_pool(name="pava", bufs=2, space="PSUM") as pava,
            tc.tile_pool(name="pavb", bufs=2, space="PSUM") as pavb,
        ):
            # ---- constants ----
            wqk_sb = cpool.tile([128, 128 * NCH], bf16, name="wqk_sb")
            for ci in range(NCH):
                nc.sync.dma_start(wqk_sb[:, ci * 128:(ci + 1) * 128],
                                  wqk_d.ap()[ci * 128:(ci + 1) * 128, :])
            bqk_sb = cpool.tile([128, 1], f32, name="bqk_sb")
            nc.sync.dma_start(bqk_sb[:], bqk_d.ap())
            wv_sb = cpool.tile([128, 512 * NCH], bf16, name="wv_sb")
            for ci in range(NCH):
                nc.gpsimd.dma_start(wv_sb[:, ci * 512:(ci + 1) * 512],
                                    wv_d.ap()[ci * 128:(ci + 1) * 128, :])

            # v_sb buffers carry a ones column at 512 (written once here):
            # the AV matmul then emits softmax row-sums as delta col 512.
            for vi in range(3):
                vinit = spool.tile([128, 513], bf16, tag="v_sb", name=f"vinit{vi}")
                nc.gpsimd.memset(vinit[:, 512:513], 1.0)

            for b in range(B):
                # ---- batch prologue: hoisted into previous batch's w-loop ----
                with tc.high_priority(offset=0 if b == 0 else 200):
                    # load slab; batch 0 goes nt-major in small pieces so the
                    # first QK group starts after ~1.6 us instead of 6.3 us
                    xs = xspool.tile([128, NCH * H * WT], bf16, tag="xs",
                                     name=f"xs{b}")
                    xs4 = xs[:].rearrange("p (c h w) -> p c h w", c=NCH, w=WT)
                    if b == 0:
                        for nt in range(4):
                            for ci in range(NCH):
                                q = nc.sync if ci < 2 else nc.scalar
                                q.dma_start(
                                    xs4[:, ci, nt * 32:(nt + 1) * 32],
                                    xa[b, ci * 128:(ci + 1) * 128,
                                       nt * 32:(nt + 1) * 32])
                    else:
                        for ci in range(NCH):
                            nc.sync.dma_start(xs4[:, ci],
                                              xa[b, ci * 128:(ci + 1) * 128])

                    # QK projection: full (h,w) range, n-tiles of 512
                    qk_sb = qkpool.tile([128, H * WT], bf16, tag="qk", name=f"qk{b}")
                    ks = qkpool.tile([64, H * WT], bf16, tag="ks", name=f"ks{b}")
                    for nt in range(H * WT // 512):
                        qkp = pqk.tile([128, 512], f32, tag="qkp")
                        for ci in range(NCH):
                            nc.tensor.matmul(
                                qkp[:],
                                wqk_sb[:, ci * 128:(ci + 1) * 128],
                                xs[:, ci * 2048 + nt * 512: ci * 2048 + (nt + 1) * 512],
                                start=(ci == 0), stop=(ci == NCH - 1))
                        if not bqk_is_zero:
                            nc.scalar.activation(qk_sb[:, nt * 512:(nt + 1) * 512],
                                                 qkp[:], AF.Identity, bias=bqk_sb[:])
                        else:
                            nc.scalar.activation(qk_sb[:, nt * 512:(nt + 1) * 512],
                                                 qkp[:], AF.Identity)
                        # K rows 64:128 -> partitions 0:63 (scores needs same base)
                        nc.sync.dma_start(ks[:, nt * 512:(nt + 1) * 512],
                                          qk_sb[64:128, nt * 512:(nt + 1) * 512])

                qk3 = qk_sb[:].rearrange("p (h w) -> p h w", w=WT)
                ks3 = ks[:].rearrange("p (h w) -> p h w", w=WT)
                sums_sb = spool.tile([128, WT], f32, tag="sums_sb", name=f"sm{b}")

                def emit_tail(prev, dcol):
                    pats, pv, pw = prev
                    base = (pw % 2) * 512
                    # delta^T(i, c) + row sums, ats stationary across both
                    ava = pava.tile([128, 256], f32, tag="ava")
                    nc.tensor.matmul(ava[:], pats[:], pv[:, 0:256],
                                     start=True, stop=True)
                    avb = pavb.tile([128, 257], f32, tag="avb")
                    nc.tensor.matmul(avb[:], pats[:], pv[:, 256:513],
                                     start=True, stop=True)
                    nc.vector.tensor_copy(dcol[:, base:base + 256], ava[:])
                    nc.vector.tensor_copy(dcol[:, base + 256:base + 512],
                                          avb[:, 0:256])
                    nc.vector.tensor_copy(sums_sb[:, pw:pw + 1], avb[:, 256:257])
                    if pw % 2 == 1:
                        nc.gpsimd.dma_start(
                            oa[b, :, pw - 1:pw + 1, :],
                            dcol[:].rearrange("p (w c) -> p w c", c=512))

                prev = None
                dcol = None
                for w in range(WT):
                    # ---- S^T(j,i) = K_w.T Q_w, bf16, k=64 ----
                    sct = psct.tile([128, 128], f32, tag="sct")
                    nc.tensor.matmul(sct[:], ks3[:, :, w], qk3[0:64, :, w],
                                     start=True, stop=True)

                    # ---- V^T_w (h, c) ----
                    vt = pvt.tile([128, 512], f32, tag="vt")
                    for ci in range(NCH):
                        nc.tensor.matmul(vt[:], xs4[:, ci, :, w],
                                         wv_sb[:, ci * 512:(ci + 1) * 512],
                                         start=(ci == 0), stop=(ci == NCH - 1))

                    # ---- attn^T (unnormalized) = exp(S^T), straight to SBUF ----
                    ats = spool.tile([128, 128], bf16, tag="ats")
                    nc.scalar.activation(ats[:], sct[:], AF.Exp)

                    # ---- V^T to SBUF bf16 (ones col at 512 persists) ----
                    v_sb = spool.tile([128, 513], bf16, tag="v_sb")
                    nc.scalar.activation(v_sb[:, 0:512], vt[:], AF.Identity)

                    # ---- AV + sums + store for previous column (PE stays fed) ----
                    if prev is not None:
                        emit_tail(prev, dcol)
                    if w % 2 == 0:
                        dcol = dcpool.tile([128, 2 * 512], bf16, tag="dcol")
                    prev = (ats, v_sb, w)

                emit_tail(prev, dcol)
                nc.sync.dma_start(sa[b], sums_sb[:])

    nc.compile()
    return nc


def kernel(x, Wq, bq, Wk, bk, Wv, bv, gamma):
    global LAST_RESULTS
    x = np.ascontiguousarray(np.asarray(x, dtype=np.float32))
    Wq = np.asarray(Wq, dtype=np.float32)
    bq = np.asarray(bq, dtype=np.float32)
    Wk = np.asarray(Wk, dtype=np.float32)
    bk = np.asarray(bk, dtype=np.float32)
    Wv = np.asarray(Wv, dtype=np.float32)
    bv = np.asarray(bv, dtype=np.float32)
    g = float(np.asarray(gamma, dtype=np.float32).reshape(-1)[0])

    nc = _build(not (np.any(bq) or np.any(bk)))

    wqkT = np.ascontiguousarray(
        np.concatenate([Wq, Wk], axis=0).T).astype(ml_dtypes.bfloat16)  # (C, 128)
    bqk = np.concatenate([bq, bk], axis=0).reshape(128, 1)
    wvT = np.ascontiguousarray((g * Wv).T).astype(ml_dtypes.bfloat16)    # (C, C)
    xb = x.astype(ml_dtypes.bfloat16)

    in_maps = []
    for k in range(N_CORES):
        in_maps.append({
            "x": np.ascontiguousarray(xb[:, :, :, k * WT:(k + 1) * WT]),
            "wqkT": wqkT,
            "bqk": bqk,
            "wvT": wvT,
        })

    res = run_bass_kernel_spmd(nc, in_maps, core_ids=list(range(N_CORES)),
                               trace=TRACE)
    LAST_RESULTS = res

    gbv = (g * bv).reshape(1, C, 1, 1).astype(np.float32)
    out = np.empty((B, C, H, W), dtype=np.float32)
    for k in range(N_CORES):
        d = np.asarray(res.results[k]["out"]).astype(np.float32)  # (B,H,WT,C)
        s = np.asarray(res.results[k]["sums"])                    # (B,H,WT)
        rr = (1.0 / s)[:, None, :, :]                             # (B,1,H,WT)
        out[:, :, :, k * WT:(k + 1) * WT] = (
            x[:, :, :, k * WT:(k + 1) * WT]
            + d.transpose(0, 3, 1, 2) * rr + gbv)
    return out


# revision 15
# speedup vs baseline: 1.1979x; 1.0215x over previous
"""ColAttention TRN2 kernel: out = gamma * colattn(x) + x.

Sharding: width. Core k gets x[:, :, :, 16k:16(k+1)] (host slice, bf16).
Per core: 8 batches x 16 width columns = 128 independent attention
problems over h=128.

v5: column-PAIR pipeline, all-bf16 (fp8 DoubleRow measured slower here —
it trips the chip power throttle to ~64% util). Per pair (w0, w1):
  S^T = K_w.T Q_w  for both columns back-to-back (128-col matmuls
        pipeline at ~56ns when adjacent; no PE transpose anywhere)
  V^T = x_w.T @ (gamma*Wv).T  8 x 512-col bf16 streams
  attn^T = exp(S^T) on ACT straight to SBUF bf16 (unnormalized;
        scores are within +-50 so exp fits f32/bf16 range)
  delta^T(i,c) = attn^T.T @ V^T  one 512-col matmul per column
  sums(i) via 1-col matmul attn^T.T @ ones, pairs share one PSUM tile
  delta pair staged to a 1024-col bf16 tile, one DMA per pair (Pool q)
Device ships unnormalized delta^T (bf16) + row sums (f32); host does
out = x + delta/sums (+ gamma*bv), keeping the residual exact in f32.
"""

import numpy as np
import ml_dtypes

import concourse.bass as bass
from concourse import bacc, mybir
from concourse.tile import TileContext
from concourse.bass_utils import run_bass_kernel_spmd

f32 = mybir.dt.float32
bf16 = mybir.dt.bfloat16
AF = mybir.ActivationFunctionType

N_CORES = 8
B, C, H, W = 8, 512, 128, 128
WT = W // N_CORES          # 16 w-columns per core
DQ = 64
NCH = C // 128             # 4 c-chunks

TRACE = False              # set True from test.py for profiling
LAST_RESULTS = None


def _build(bqk_is_zero: bool):
    nc = bacc.Bacc("TRN2", num_devices=N_CORES, debug=False)

    x_d = nc.dram_tensor("x", (B, C, H, WT), bf16, kind="ExternalInput")
    wqk_d = nc.dram_tensor("wqkT", (C, 128), bf16, kind="ExternalInput")
    bqk_d = nc.dram_tensor("bqk", (128, 1), f32, kind="ExternalInput")
    wv_d = nc.dram_tensor("wvT", (C, C), bf16, kind="ExternalInput")
    out_d = nc.dram_tensor("out", (B, H, WT, C), bf16, kind="ExternalOutput")
    sums_d = nc.dram_tensor("sums", (B, H, WT), f32, kind="ExternalOutput")
    ones_d = nc.inline_tensor(np.ones((128, 1), dtype=ml_dtypes.bfloat16),
                              name="ones128")

    xa = x_d.ap()
    oa = out_d.ap()
    sa = sums_d.ap()

    with TileContext(nc) as tc:
        with (
            tc.tile_pool(name="const", bufs=1) as cpool,
            tc.tile_pool(name="xs", bufs=2) as xspool,
            tc.tile_pool(name="qk", bufs=2) as qkpool,
            tc.tile_pool(name="small", bufs=4) as spool,
            tc.tile_pool(name="dc", bufs=2) as dcpool,
            tc.tile_pool(name="pqk", bufs=1, space="PSUM") as pqk,
            tc.tile_pool(name="pvt", bufs=2, space="PSUM") as pvt,
            tc.tile_pool(name="psct", bufs=2, space="PSUM") as psct,
            tc.tile_pool(name="pav", bufs=1, space="PSUM") as pav,
            tc.tile_pool(name="psm", bufs=1, space="PSUM") as psm,
        ):
            # ---- constants ----
            wqk_sb = cpool.tile([128, 128 * NCH], bf16, name="wqk_sb")
            for ci in range(NCH):
                nc.sync.dma_start(wqk_sb[:, ci * 128:(ci + 1) * 128],
                                  wqk_d.ap()[ci * 128:(ci + 1) * 128, :])
            bqk_sb = cpool.tile([128, 1], f32, name="bqk_sb")
            nc.sync.dma_start(bqk_sb[:], bqk_d.ap())
            ones_sb = cpool.tile([128, 1], bf16, name="ones_sb")
            nc.sync.dma_start(ones_sb[:], ones_d.ap())
            wv_sb = cpool.tile([128, 512 * NCH], bf16, name="wv_sb")
            for ci in range(NCH):
                nc.gpsimd.dma_start(wv_sb[:, ci * 512:(ci + 1) * 512],
                                    wv_d.ap()[ci * 128:(ci + 1) * 128, :])

            for b in range(B):
                # ---- batch prologue: hoisted into previous batch's w-loop ----
                with tc.high_priority(offset=0 if b == 0 else 200):
                    # load slab; batch 0 goes nt-major in small pieces so the
                    # first QK group starts after ~2 us instead of ~7 us
                    xs = xspool.tile([128, NCH * H * WT], bf16, tag="xs",
                                     name=f"xs{b}")
                    xs4 = xs[:].rearrange("p (c h w) -> p c h w", c=NCH, w=WT)
                    if b == 0:
                        for nt in range(4):
                            for ci in range(NCH):
                                q = nc.sync if ci < 2 else nc.scalar
                                q.dma_start(
                                    xs4[:, ci, nt * 32:(nt + 1) * 32],
                                    xa[b, ci * 128:(ci + 1) * 128,
                                       nt * 32:(nt + 1) * 32])
                    else:
                        for ci in range(NCH):
                            nc.sync.dma_start(xs4[:, ci],
                                              xa[b, ci * 128:(ci + 1) * 128])

                    # QK projection: full (h,w) range, n-tiles of 512
                    qk_sb = qkpool.tile([128, H * WT], bf16, tag="qk", name=f"qk{b}")
                    ks = qkpool.tile([64, H * WT], bf16, tag="ks", name=f"ks{b}")
                    for nt in range(H * WT // 512):
                        qkp = pqk.tile([128, 512], f32, tag="qkp")
                        for ci in range(NCH):
                            nc.tensor.matmul(
                                qkp[:],
                                wqk_sb[:, ci * 128:(ci + 1) * 128],
                                xs[:, ci * 2048 + nt * 512: ci * 2048 + (nt + 1) * 512],
                                start=(ci == 0), stop=(ci == NCH - 1))
                        if not bqk_is_zero:
                            nc.scalar.activation(qk_sb[:, nt * 512:(nt + 1) * 512],
                                                 qkp[:], AF.Identity, bias=bqk_sb[:])
                        else:
                            nc.vector.tensor_copy(qk_sb[:, nt * 512:(nt + 1) * 512],
                                                  qkp[:])
                        # K rows 64:128 -> partitions 0:63 (scores needs same base)
                        nc.sync.dma_start(ks[:, nt * 512:(nt + 1) * 512],
                                          qk_sb[64:128, nt * 512:(nt + 1) * 512])

                qk3 = qk_sb[:].rearrange("p (h w) -> p h w", w=WT)
                ks3 = ks[:].rearrange("p (h w) -> p h w", w=WT)
                sums_sb = spool.tile([128, WT], f32, tag="sums_sb", name=f"sm{b}")

                def emit_tail(prev):
                    # AV + sums for a finished pair; 1-col sums matmuls are
                    # sandwiched between 512-col AV streams to hide pipe fill
                    (a0, v0, w0), (a1, v1, w1) = prev
                    smp = psm.tile([128, 2], f32, tag="smp")
                    dcol = dcpool.tile([128, 2 * 512], bf16, tag="dcol")
                    av0 = pav.tile([128, 512], f32, tag="av0")
                    nc.tensor.matmul(av0[:], a0[:], v0[:], start=True, stop=True)
                    nc.tensor.matmul(smp[:, 0:1], a0[:], ones_sb[:],
                                     start=True, stop=True, skip_group_check=True)
                    av1 = pav.tile([128, 512], f32, tag="av1")
                    nc.tensor.matmul(av1[:], a1[:], v1[:], start=True, stop=True)
                    nc.tensor.matmul(smp[:, 1:2], a1[:], ones_sb[:],
                                     start=True, stop=True, skip_group_check=True)
                    nc.vector.tensor_copy(dcol[:, 0:512], av0[:])
                    nc.vector.tensor_copy(dcol[:, 512:1024], av1[:])
                    nc.vector.tensor_copy(sums_sb[:, w0:w0 + 2], smp[:])
                    nc.gpsimd.dma_start(
                        oa[b, :, w0:w0 + 2, :],
                        dcol[:].rearrange("p (w c) -> p w c", c=512))

                prev = None
                for w0 in range(0, WT, 2):
                    cur = []
                    for w in (w0, w0 + 1):
                        # ---- S^T(j,i) = K_w.T Q_w, bf16, k=64 ----
                        sct = psct.tile([128, 128], f32, tag="sct")
                        nc.tensor.matmul(sct[:], ks3[:, :, w], qk3[0:64, :, w],
                                         start=True, stop=True)
                        cur.append(sct)
                    for wi, w in enumerate((w0, w0 + 1)):
                        # ---- V^T_w (h, c) ----
                        vt = pvt.tile([128, 512], f32, tag="vt")
                        for ci in range(NCH):
                            nc.tensor.matmul(vt[:], xs4[:, ci, :, w],
                                             wv_sb[:, ci * 512:(ci + 1) * 512],
                                             start=(ci == 0), stop=(ci == NCH - 1))
                        # ---- attn^T (unnormalized), straight to SBUF ----
                        ats = spool.tile([128, 128], bf16, tag="ats")
                        nc.scalar.activation(ats[:], cur[wi][:], AF.Exp)
                        # ---- V^T to SBUF bf16 ----
                        v_sb = spool.tile([128, 512], bf16, tag="v_sb")
                        nc.scalar.activation(v_sb[:], vt[:], AF.Identity)
                        cur[wi] = (ats, v_sb, w)

                    if prev is not None:
                        emit_tail(prev)
                    prev = cur

                emit_tail(prev)
                nc.sync.dma_start(sa[b], sums_sb[:])

    nc.compile()
    return nc


def kernel(x, Wq, bq, Wk, bk, Wv, bv, gamma):
    global LAST_RESULTS
    x = np.ascontiguousarray(np.asarray(x, dtype=np.float32))
    Wq = np.asarray(Wq, dtype=np.float32)
    bq = np.asarray(bq, dtype=np.float32)
    Wk = np.asarray(Wk, dtype=np.float32)
    bk = np.asarray(bk, dtype=np.float32)
    Wv = np.asarray(Wv, dtype=np.float32)
    bv = np.asarray(bv, dtype=np.float32)
    g = float(np.asarray(gamma, dtype=np.float32).reshape(-1)[0])

    nc = _build(not (np.any(bq) or np.any(bk)))

    wqkT = np.ascontiguousarray(
        np.concatenate([Wq, Wk], axis=0).T).astype(ml_dtypes.bfloat16)  # (C, 128)
    bqk = np.concatenate([bq, bk], axis=0).reshape(128, 1)
    wvT = np.ascontiguousarray((g * Wv).T).astype(ml_dtypes.bfloat16)    # (C, C)
    xb = x.astype(ml_dtypes.bfloat16)

    in_maps = []
    for k in range(N_CORES):
        in_maps.append({
            "x": np.ascontiguousarray(xb[:, :, :, k * WT:(k + 1) * WT]),
            "wqkT": wqkT,
            "bqk": bqk,
            "wvT": wvT,
        })

    res = run_bass_kernel_spmd(nc, in_maps, core_ids=list(range(N_CORES)),
                               trace=TRACE)
    LAST_RESULTS = res

    gbv = (g * bv).reshape(1, C, 1, 1).astype(np.float32)
    out = np.empty((B, C, H, W), dtype=np.float32)
    for k in range(N_CORES):
        d = np.asarray(res.results[k]["out"]).astype(np.float32)  # (B,H,WT,C)
        s = np.asarray(res.results[k]["sums"])                    # (B,H,WT)
        rr = (1.0 / s)[:, None, :, :]                             # (B,1,H,WT)
        out[:, :, :, k * WT:(k + 1) * WT] = (
            x[:, :, :, k * WT:(k + 1) * WT]
            + d.transpose(0, 3, 1, 2) * rr + gbv)
    return out


# revision 20
# speedup vs baseline: 1.3213x; 1.1030x over previous
"""ColAttention TRN2 kernel: out = gamma * colattn(x) + x.

Sharding: width. Core k gets x[:, :, :, 16k:16(k+1)] (host slice, bf16).
Per core: 8 batches x 16 width columns = 128 independent attention
problems over h=128.

v5: column-PAIR pipeline, all-bf16 (fp8 DoubleRow measured slower here —
it trips the chip power throttle to ~64% util). Per pair (w0, w1):
  S^T = K_w.T Q_w  for both columns back-to-back (128-col matmuls
        pipeline at ~56ns when adjacent; no PE transpose anywhere)
  V^T = x_w.T @ (gamma*Wv).T  8 x 512-col bf16 streams
  attn^T = exp(S^T) on ACT straight to SBUF bf16 (unnormalized;
        scores are within +-50 so exp fits f32/bf16 range)
  delta^T(i,c) = attn^T.T @ V^T  one 512-col matmul per column
  sums(i) via 1-col matmul attn^T.T @ ones, pairs share one PSUM tile
  delta pair staged to a 1024-col bf16 tile, one DMA per pair (Pool q)
Device ships unnormalized delta^T (bf16) + row sums (f32); host does
out = x + delta/sums (+ gamma*bv), keeping the residual exact in f32.
"""

import numpy as np
import ml_dtypes

import concourse.bass as bass
from concourse import bacc, mybir
from concourse.tile import TileContext
from concourse.bass_utils import run_bass_kernel_spmd

f32 = mybir.dt.float32
bf16 = mybir.dt.bfloat16
AF = mybir.ActivationFunctionType

N_CORES = 8
B, C, H, W = 8, 512, 128, 128
WT = W // N_CORES          # 16 w-columns per core
DQ = 64
NCH = C // 128             # 4 c-chunks

TRACE = False              # set True from test.py for profiling
LAST_RESULTS = None


def _build(bqk_is_zero: bool):
    nc = bacc.Bacc("TRN2", num_devices=N_CORES, debug=False)

    x_d = nc.dram_tensor("x", (B, C, WT, H), bf16, kind="ExternalInput")
    wqk_d = nc.dram_tensor("wqkT", (C, 128), bf16, kind="ExternalInput")
    bqk_d = nc.dram_tensor("bqk", (128, 1), f32, kind="ExternalInput")
    wv_d = nc.dram_tensor("wvT", (C, C), bf16, kind="ExternalInput")
    out_d = nc.dram_tensor("out", (B, H, WT, C), bf16, kind="ExternalOutput")
    sums_d = nc.dram_tensor("sums", (B, H, WT), f32, kind="ExternalOutput")
    ones_d = nc.inline_tensor(np.ones((128, 1), dtype=ml_dtypes.bfloat16),
                              name="ones128")

    xa = x_d.ap()
    oa = out_d.ap()
    sa = sums_d.ap()

    with TileContext(nc) as tc:
        with (
            tc.tile_pool(name="const", bufs=1) as cpool,
            tc.tile_pool(name="xs", bufs=2) as xspool,
            tc.tile_pool(name="qk", bufs=2) as qkpool,
            tc.tile_pool(name="small", bufs=4) as spool,
            tc.tile_pool(name="dc", bufs=2) as dcpool,
            tc.tile_pool(name="pqk", bufs=1, space="PSUM") as pqk,
            tc.tile_pool(name="pvt", bufs=2, space="PSUM") as pvt,
            tc.tile_pool(name="psct", bufs=2, space="PSUM") as psct,
            tc.tile_pool(name="pav", bufs=1, space="PSUM") as pav,
            tc.tile_pool(name="psm", bufs=1, space="PSUM") as psm,
        ):
            # ---- constants ----
            wqk_sb = cpool.tile([128, 128 * NCH], bf16, name="wqk_sb")
            for ci in range(NCH):
                nc.sync.dma_start(wqk_sb[:, ci * 128:(ci + 1) * 128],
                                  wqk_d.ap()[ci * 128:(ci + 1) * 128, :])
            bqk_sb = cpool.tile([128, 1], f32, name="bqk_sb")
            nc.sync.dma_start(bqk_sb[:], bqk_d.ap())
            ones_sb = cpool.tile([128, 1], bf16, name="ones_sb")
            nc.sync.dma_start(ones_sb[:], ones_d.ap())
            wv_sb = cpool.tile([128, 512 * NCH], bf16, name="wv_sb")
            for ci in range(NCH):
                nc.gpsimd.dma_start(wv_sb[:, ci * 512:(ci + 1) * 512],
                                    wv_d.ap()[ci * 128:(ci + 1) * 128, :])

            for b in range(B):
                # ---- batch prologue: hoisted into previous batch's w-loop ----
                with tc.high_priority(offset=0 if b == 0 else 200):
                    # load slab; batch 0 goes nt-major in small pieces so the
                    # first QK group starts after ~2 us instead of ~7 us
                    xs = xspool.tile([128, NCH * H * WT], bf16, tag="xs",
                                     name=f"xs{b}")
                    xs4 = xs[:].rearrange("p (c w h) -> p c w h", c=NCH, w=WT)
                    if b == 0:
                        for nt in range(4):
                            for ci in range(NCH):
                                q = nc.sync if ci < 2 else nc.scalar
                                q.dma_start(
                                    xs4[:, ci, nt * 4:(nt + 1) * 4],
                                    xa[b, ci * 128:(ci + 1) * 128,
                                       nt * 4:(nt + 1) * 4])
                    else:
                        for ci in range(NCH):
                            nc.sync.dma_start(xs4[:, ci],
                                              xa[b, ci * 128:(ci + 1) * 128])

                    # QK projection: full (h,w) range, n-tiles of 512
                    qk_sb = qkpool.tile([128, H * WT], bf16, tag="qk", name=f"qk{b}")
                    ks = qkpool.tile([64, H * WT], bf16, tag="ks", name=f"ks{b}")
                    for nt in range(H * WT // 512):
                        qkp = pqk.tile([128, 512], f32, tag="qkp")
                        for ci in range(NCH):
                            nc.tensor.matmul(
                                qkp[:],
                                wqk_sb[:, ci * 128:(ci + 1) * 128],
                                xs[:, ci * 2048 + nt * 512: ci * 2048 + (nt + 1) * 512],
                                start=(ci == 0), stop=(ci == NCH - 1))
                        if not bqk_is_zero:
                            nc.scalar.activation(qk_sb[:, nt * 512:(nt + 1) * 512],
                                                 qkp[:], AF.Identity, bias=bqk_sb[:])
                        else:
                            nc.vector.tensor_copy(qk_sb[:, nt * 512:(nt + 1) * 512],
                                                  qkp[:])
                        # K rows 64:128 -> partitions 0:63 (scores needs same base)
                        nc.sync.dma_start(ks[:, nt * 512:(nt + 1) * 512],
                                          qk_sb[64:128, nt * 512:(nt + 1) * 512])

                qk3 = qk_sb[:].rearrange("p (w h) -> p w h", w=WT)
                ks3 = ks[:].rearrange("p (w h) -> p w h", w=WT)
                sums_sb = spool.tile([128, WT], f32, tag="sums_sb", name=f"sm{b}")

                def emit_tail(prev):
                    # AV + sums for a finished pair; 1-col sums matmuls are
                    # sandwiched between 512-col AV streams to hide pipe fill
                    (a0, v0, w0), (a1, v1, w1) = prev
                    smp = psm.tile([128, 2], f32, tag="smp")
                    dcol = dcpool.tile([128, 2 * 512], bf16, tag="dcol")
                    av0 = pav.tile([128, 512], f32, tag="av0")
                    nc.tensor.matmul(av0[:], a0[:], v0[:], start=True, stop=True)
                    nc.tensor.matmul(smp[:, 0:1], a0[:], ones_sb[:],
                                     start=True, stop=True, skip_group_check=True)
                    av1 = pav.tile([128, 512], f32, tag="av1")
                    nc.tensor.matmul(av1[:], a1[:], v1[:], start=True, stop=True)
                    nc.tensor.matmul(smp[:, 1:2], a1[:], ones_sb[:],
                                     start=True, stop=True, skip_group_check=True)
                    nc.vector.tensor_copy(dcol[:, 0:512], av0[:])
                    nc.vector.tensor_copy(dcol[:, 512:1024], av1[:])
                    nc.vector.tensor_copy(sums_sb[:, w0:w0 + 2], smp[:])
                    nc.gpsimd.dma_start(
                        oa[b, :, w0:w0 + 2, :],
                        dcol[:].rearrange("p (w c) -> p w c", c=512))

                prev = None
                for w0 in range(0, WT, 2):
                    cur = []
                    for w in (w0, w0 + 1):
                        # ---- S^T(j,i) = K_w.T Q_w, bf16, k=64, contiguous ----
                        sct = psct.tile([128, 128], f32, tag="sct")
                        nc.tensor.matmul(sct[:], ks3[:, w], qk3[0:64, w],
                                         start=True, stop=True)
                        cur.append(sct)
                    for wi, w in enumerate((w0, w0 + 1)):
                        # ---- V^T_w (h, c) ----
                        vt = pvt.tile([128, 512], f32, tag="vt")
                        for ci in range(NCH):
                            nc.tensor.matmul(vt[:], xs4[:, ci, w],
                                             wv_sb[:, ci * 512:(ci + 1) * 512],
                                             start=(ci == 0), stop=(ci == NCH - 1))
                        # ---- attn^T (unnormalized), straight to SBUF ----
                        ats = spool.tile([128, 128], bf16, tag="ats")
                        nc.scalar.activation(ats[:], cur[wi][:], AF.Exp)
                        # ---- V^T to SBUF bf16 (split engines to avoid backlog) ----
                        v_sb = spool.tile([128, 512], bf16, tag="v_sb")
                        if wi == 1:
                            nc.scalar.activation(v_sb[:], vt[:], AF.Identity)
                        else:
                            nc.vector.tensor_copy(v_sb[:], vt[:])
                        cur[wi] = (ats, v_sb, w)

                    if prev is not None:
                        emit_tail(prev)
                    prev = cur

                emit_tail(prev)
                nc.sync.dma_start(sa[b], sums_sb[:])

    nc.compile()
    return nc


def kernel(x, Wq, bq, Wk, bk, Wv, bv, gamma):
    global LAST_RESULTS
    x = np.ascontiguousarray(np.asarray(x, dtype=np.float32))
    Wq = np.asarray(Wq, dtype=np.float32)
    bq = np.asarray(bq, dtype=np.float32)
    Wk = np.asarray(Wk, dtype=np.float32)
    bk = np.asarray(bk, dtype=np.float32)
    Wv = np.asarray(Wv, dtype=np.float32)
    bv = np.asarray(bv, dtype=np.float32)
    g = float(np.asarray(gamma, dtype=np.float32).reshape(-1)[0])

    nc = _build(not (np.any(bq) or np.any(bk)))

    wqkT = np.ascontiguousarray(
        np.concatenate([Wq, Wk], axis=0).T).astype(ml_dtypes.bfloat16)  # (C, 128)
    bqk = np.concatenate([bq, bk], axis=0).reshape(128, 1)
    wvT = np.ascontiguousarray((g * Wv).T).astype(ml_dtypes.bfloat16)    # (C, C)
    xb = x.astype(ml_dtypes.bfloat16)

    in_maps = []
    for k in range(N_CORES):
        in_maps.append({
            "x": np.ascontiguousarray(
                xb[:, :, :, k * WT:(k + 1) * WT].transpose(0, 1, 3, 2)),
            "wqkT": wqkT,
            "bqk": bqk,
            "wvT": wvT,
        })

    res = run_bass_kernel_spmd(nc, in_maps, core_ids=list(range(N_CORES)),
                               trace=TRACE)
    LAST_RESULTS = res

    gbv = (g * bv).reshape(1, C, 1, 1).astype(np.float32)
    out = np.empty((B, C, H, W), dtype=np.float32)
    for k in range(N_CORES):
        d = np.asarray(res.results[k]["out"]).astype(np.float32)  # (B,H,WT,C)
        s = np.asarray(res.results[k]["sums"])                    # (B,H,WT)
        rr = (1.0 / s)[:, None, :, :]                             # (B,1,H,WT)
        out[:, :, :, k * WT:(k + 1) * WT] = (
            x[:, :, :, k * WT:(k + 1) * WT]
            + d.transpose(0, 3, 1, 2) * rr + gbv)
    return out
